# revision 1
# baseline (speedup 1.0000x reference)
"""Trainium2 Bass kernel for ragged multi-head attention (B=16,S=512,H=1024,NH=16).

Sharding: data-parallel over batch — 16 samples over 8 cores, 2 samples/core.
Per core, everything is computed in a "transposed activation" layout so no
on-chip transposes are ever needed:

  - host passes x pre-transposed per core:        xT   [H, T=1024]  (tokens of 2 samples)
  - host passes weights pre-transposed:           w*T  [H_in, H_out]
  - Q^T, K^T projections produce [H_out, T]       (partition = head dim!)
  - scores^T[k,q] = (K^T_h chunk).T @ Q^T_h       (contraction d=64 on partitions;
    odd/even heads sit at partition offsets 0/64, so a head-pair's score
    matmuls run concurrently in disjoint PE row groups)
  - softmax without max-subtraction (scores are O(1)); the key mask folds into
    the exp's per-partition bias; row-sums come free from a ones column
    appended to V ("V_aug"), computed by the same PV matmul
  - out^T_h[d,q] = V_aug.T @ P^T ; row 64 = softmax denominators
  - normalization: reciprocal (DVE) -> broadcast across 64 partitions
    (GPSIMD partition_broadcast) -> multiply (DVE)
  - fc output is computed in natural [t, o] layout (attnout^T chunks are the
    stationary operand), so the result DMAs out contiguously.

Ragged: per-sample lengths known on host; samples sorted by length into 2
"slots" (slot n-of-tiles = max over the 8 cores' samples in that slot), and all
per-token loops only cover ceil(L/128) 128-token tiles. Rows at/after cls_len
are exactly `bf` in the reference and are filled on the host.
"""

import math

import numpy as np

B, S, H, NH, DH = 16, 512, 1024, 16, 64
N_CORES = 8
B_LOC = B // N_CORES  # 2
T = B_LOC * S  # 1024 tokens per core
P = 128
G = H // P  # 8 contraction chunks
NEG = -30.0  # additive bias for masked keys: exp(-30) ~ 9e-14

_CACHE = {}


def _build(ns, loop_iters=None, dtcfg=None):
    """Build the Bass program for per-slot tile counts ns=(n0,n1).

    loop_iters: if not None, wrap the whole body in a hardware For_i loop
    (used only for benchmarking).  Returns the compiled nc.
    """
    import concourse.bass as bass
    import concourse.mybir as mybir
    import concourse.tile as tile
    from concourse import bacc

    dt = mybir.dt
    cfg = dict(
        qk=dt.bfloat16,   # xT / wq / wk / wv / Q^T / K^T storage + matmul dtype
        pv=dt.bfloat16,   # P^T / V_aug storage + PV matmul dtype
        fc=dt.bfloat16,   # attnout^T / wfT storage + FC matmul dtype
    )
    if dtcfg:
        cfg.update(dtcfg)
    FQK = cfg["qk"]
    FPV = cfg["pv"]
    FFC = cfg["fc"]

    nc = bacc.Bacc("TRN2", target_bir_lowering=False, debug=False,
                   num_devices=N_CORES)

    f32 = dt.float32
    f32r = dt.float32r
    xT = nc.dram_tensor("xT", [H, T], FQK, kind="ExternalInput")
    # wq/wk host-swizzled to [j, p, g, c]: column-block j is one contiguous
    # 256KB read with 2KB runs (strided reads of 256B pay a 2x DMA penalty)
    wqT = nc.dram_tensor("wqT", [G, P, G, P], FQK, kind="ExternalInput")
    wkT = nc.dram_tensor("wkT", [G, P, G, P], FQK, kind="ExternalInput")
    wvT = nc.dram_tensor("wvT", [H, H], FQK, kind="ExternalInput")
    wfT = nc.dram_tensor("wfT", [H, H], FFC, kind="ExternalInput")
    bq2 = nc.dram_tensor("bq2", [G, P], f32, kind="ExternalInput")
    bk2 = nc.dram_tensor("bk2", [G, P], f32, kind="ExternalInput")
    bv1 = nc.dram_tensor("bv1", [1, H], f32, kind="ExternalInput")
    bf1 = nc.dram_tensor("bf1", [1, H], f32, kind="ExternalInput")
    kbias = nc.dram_tensor("kbias", [B_LOC, 4, P], f32, kind="ExternalInput")
    y = nc.dram_tensor("y", [T, H], f32, kind="ExternalOutput")

    HA = 65  # per-head V columns incl. ones column

    with tile.TileContext(nc) as tc:
        import contextlib
        ctx = contextlib.ExitStack()
        with ctx:
            const = ctx.enter_context(tc.tile_pool(name="const", bufs=1))
            wqk_pool = ctx.enter_context(tc.tile_pool(name="wqk", bufs=3))
            pt_pool = ctx.enter_context(tc.tile_pool(name="pt", bufs=14))
            r_pool = ctx.enter_context(tc.tile_pool(name="r", bufs=4))
            rb_pool = ctx.enter_context(tc.tile_pool(name="rb", bufs=4))
            out_pool = ctx.enter_context(tc.tile_pool(name="out", bufs=4))
            ps_mm = ctx.enter_context(tc.tile_pool(name="psmm", bufs=3, space="PSUM"))
            ps_sc = ctx.enter_context(tc.tile_pool(name="pssc", bufs=3, space="PSUM"))
            ps_pv = ctx.enter_context(tc.tile_pool(name="pspv", bufs=2, space="PSUM"))

            # resident tensors
            xT_sb = const.tile([P, G, T], FQK)
            qT_sb = const.tile([P, G, T], FQK)
            kT_sb = const.tile([P, G, T], FQK)
            vaug_sb = const.tile([P, 2 * 4, NH * HA], FPV)
            aoT_sb = const.tile([P, G, T], FFC)
            wv_sb = const.tile([P, G, H], FQK)
            wf_sb = const.tile([P, G, H], FFC)
            bq_sb = const.tile([P, G], f32)
            bk_sb = const.tile([P, G], f32)
            bv_sb = const.tile([P, H], f32)
            bf_sb = const.tile([P, H], f32)
            kb_sb = const.tile([P, B_LOC, 4], f32)

            def body():
                # ---- preload ----
                # xT chunks alternate between the SP and ACT HWDGE queues so
                # the first compute-gating load finishes sooner; everything
                # else the first matmuls don't need goes on the ACT queue.
                for g in range(G):
                    eng = nc.sync if g % 2 == 0 else nc.scalar
                    eng.dma_start(
                        out=xT_sb[:, g, :],
                        in_=xT.ap()[g * P:(g + 1) * P, :])
                nc.scalar.dma_start(
                    out=bq_sb[:], in_=bq2.ap().rearrange("g p -> p g"))
                nc.scalar.dma_start(
                    out=bk_sb[:], in_=bk2.ap().rearrange("g p -> p g"))
                bvap = bv1.ap()
                nc.scalar.dma_start(
                    out=bv_sb[:],
                    in_=bass.AP(tensor=bvap.tensor, offset=bvap.offset,
                                ap=[[0, P]] + list(bvap.ap[1:])))
                bfap = bf1.ap()
                nc.scalar.dma_start(
                    out=bf_sb[:],
                    in_=bass.AP(tensor=bfap.tensor, offset=bfap.offset,
                                ap=[[0, P]] + list(bfap.ap[1:])))
                nc.scalar.dma_start(
                    out=kb_sb[:], in_=kbias.ap().rearrange("s k p -> p s k"))
                nc.scalar.dma_start(
                    out=wv_sb[:],
                    in_=wvT.ap().rearrange("(g p) o -> p g o", p=P))
                nc.scalar.dma_start(
                    out=wf_sb[:],
                    in_=wfT.ap().rearrange("(g p) o -> p g o", p=P))
                # ones columns of V_aug
                nc.vector.memset(
                    vaug_sb.rearrange("p t (h c) -> p t h c", c=HA)[:, :, :, 64:65],
                    1.0)

                # ---- Q^T / K^T projections ----
                # weight column-block j is loaded once; both slots' accumulation
                # chains are interleaved per g so each stationary load serves 2
                # matmuls.
                for wT_d, b_sb, dst in ((wqT, bq_sb, qT_sb), (wkT, bk_sb, kT_sb)):
                    for j in range(G):
                        wblk = wqk_pool.tile([P, G, P], FQK, tag="wqk")
                        nc.sync.dma_start(out=wblk[:], in_=wT_d.ap()[j])
                        pss = {}
                        for s in range(B_LOC):
                            if ns[s]:
                                pss[s] = ps_mm.tile([P, 512], f32, tag="psmm", name=f"psqk{s}")
                        for g in range(G):
                            for s, ps in pss.items():
                                nc.tensor.matmul(
                                    ps[:, :ns[s] * P], lhsT=wblk[:, g, :],
                                    rhs=xT_sb[:, g, s * S:s * S + ns[s] * P],
                                    start=(g == 0), stop=(g == G - 1))
                        for s, ps in pss.items():
                            W = ns[s] * P
                            nc.vector.tensor_scalar_add(
                                out=dst[:, j, s * S:s * S + W], in0=ps[:, :W],
                                scalar1=b_sb[:, j:j + 1])

                # ---- V projection (natural layout, into V_aug) ----
                for ob in range(2):  # 512-wide output column blocks
                    for s in range(B_LOC):
                        for tt in range(ns[s]):
                            tg = s * 4 + tt
                            ps = ps_mm.tile([P, 512], f32, tag="psmm")
                            for g in range(G):
                                nc.tensor.matmul(
                                    ps[:],
                                    lhsT=xT_sb[:, g, tg * P:(tg + 1) * P],
                                    rhs=wv_sb[:, g, ob * 512:(ob + 1) * 512],
                                    start=(g == 0), stop=(g == G - 1))
                            vdst = vaug_sb[:, tg, :].rearrange(
                                "p (h c) -> p h c", c=HA)[:, ob * 8:(ob + 1) * 8, 0:64]
                            nc.vector.tensor_add(
                                vdst,
                                ps[:].rearrange("p (h c) -> p h c", c=64),
                                bv_sb[:, ob * 512:(ob + 1) * 512].rearrange(
                                    "p (h c) -> p h c", c=64))

                # ---- attention, head-pairs share PE row groups ----
                def attn_slot(s):
                    n = ns[s]
                    if n == 0:
                        return
                    W = n * P
                    s0 = s * S
                    for g2 in range(G):
                        pts = {0: [], 1: []}
                        for kt in range(n):
                            for hh in (0, 1):
                                po = hh * 64
                                ps = ps_sc.tile([P, 512], f32, tag="pssc")
                                nc.tensor.matmul(
                                    ps[:, :W],
                                    lhsT=kT_sb[po:po + 64, g2,
                                               s0 + kt * P:s0 + (kt + 1) * P],
                                    rhs=qT_sb[po:po + 64, g2, s0:s0 + W],
                                    start=True, stop=True)
                                pt = pt_pool.tile([P, 512], FPV, tag="pt")
                                nc.scalar.activation(
                                    out=pt[:, :W], in_=ps[:, :W],
                                    func=mybir.ActivationFunctionType.Exp,
                                    bias=kb_sb[:, s, kt:kt + 1], scale=0.125)
                                pts[hh].append(pt)
                        for hh in (0, 1):
                            h = 2 * g2 + hh
                            po = hh * 64
                            pv = ps_pv.tile([HA, 512], f32, tag="pspv")
                            for kt in range(n):
                                nc.tensor.matmul(
                                    pv[:, :W],
                                    lhsT=vaug_sb[:, s * 4 + kt,
                                                 HA * h:HA * h + HA],
                                    rhs=pts[hh][kt][:, :W],
                                    start=(kt == 0), stop=(kt == n - 1))
                            r = r_pool.tile([1, 512], f32, tag="r")
                            nc.vector.reciprocal(out=r[:, :W],
                                                 in_=pv[64:65, :W])
                            # broadcast r across 64 partitions on the (idle)
                            # GPSIMD engine
                            rb = rb_pool.tile([64, 512], f32, tag="rb")
                            nc.gpsimd.partition_broadcast(rb[:, :W], r[:, :W])
                            nc.vector.tensor_mul(
                                aoT_sb[po:po + 64, g2, s0:s0 + W],
                                pv[0:64, :W], rb[:, :W])

                # ---- FC (per slot so the small slot's fc overlaps the
                # big slot's attention) ----
                def fc_slot(s):
                    for oh in range(2):
                        for tt in range(ns[s]):
                            tg = s * 4 + tt
                            ps = ps_mm.tile([P, 512], f32, tag="psmm")
                            for g in range(G):
                                nc.tensor.matmul(
                                    ps[:],
                                    lhsT=aoT_sb[:, g, tg * P:(tg + 1) * P],
                                    rhs=wf_sb[:, g, oh * 512:(oh + 1) * 512],
                                    start=(g == 0), stop=(g == G - 1))
                            ot = out_pool.tile([P, 512], f32, tag="out")
                            nc.vector.tensor_add(
                                ot[:], ps[:], bf_sb[:, oh * 512:(oh + 1) * 512])
                            nc.gpsimd.dma_start(
                                out=y.ap()[tg * P:(tg + 1) * P,
                                           oh * 512:(oh + 1) * 512],
                                in_=ot[:])

                # small slot first: its fc fills PE gaps during the big
                # slot's attention; only the big slot's fc forms the tail
                slot_order = sorted(range(B_LOC), key=lambda s: ns[s])
                for s in slot_order:
                    attn_slot(s)
                    fc_slot(s)

            if loop_iters is None:
                body()
            else:
                # benchmark-only loop; prefetch hints keep the back-edge
                # branch from paying an IRAM refetch on every iteration
                with tc.For_i(0, loop_iters, 1,
                              hint_engines=tuple(mybir.ALL_ENGINES)):
                    body()

    nc.compile()
    return nc


def _make_runner(nc):
    """Compile nc into a reusable 8-core jitted callable (axon PJRT path)."""
    import jax
    import numpy as _np
    from jax.experimental.shard_map import shard_map
    from jax.sharding import Mesh, NamedSharding, PartitionSpec

    import concourse.mybir as mybir
    from concourse import bass2jax

    bass2jax.install_neuronx_cc_hook()
    partition_name = (nc.partition_id_tensor.name
                      if nc.partition_id_tensor else None)
    in_names, out_names, out_avals, zero_outs = [], [], [], []
    for alloc in nc.m.functions[0].allocations:
        if not isinstance(alloc, mybir.MemoryLocationSet):
            continue
        name = alloc.memorylocations[0].name
        if alloc.kind == "ExternalInput":
            if name != partition_name:
                in_names.append(name)
        elif alloc.kind == "ExternalOutput":
            shape = tuple(alloc.tensor_shape)
            dtype = mybir.dt.np(alloc.dtype)
            out_names.append(name)
            out_avals.append(jax.core.ShapedArray(shape, dtype))
            zero_outs.append(_np.zeros(shape, dtype))
    n_params = len(in_names)
    in_names_all = in_names + out_names
    if partition_name is not None:
        in_names_all.append(partition_name)

    def _body(*args):
        operands = list(args)
        if partition_name is not None:
            operands.append(bass2jax.partition_id_tensor())
        outs = bass2jax._bass_exec_p.bind(
            *operands, out_avals=tuple(out_avals),
            in_names=tuple(in_names_all), out_names=tuple(out_names),
            lowering_input_output_aliases=(),
            sim_require_finite=True, sim_require_nnan=True, nc=nc)
        return tuple(outs)

    devices = jax.devices()[:N_CORES]
    mesh = Mesh(np.asarray(devices), ("core",))
    nio = n_params + len(out_names)
    sharded = jax.jit(
        shard_map(_body, mesh=mesh,
                  in_specs=(PartitionSpec("core"),) * nio,
                  out_specs=(PartitionSpec("core"),) * len(out_names),
                  check_rep=False),
        keep_unused=True)
    sharding = NamedSharding(mesh, PartitionSpec("core"))

    def stage(in_maps):
        per_core = [[_np.asarray(m[nm]) for nm in in_names] for m in in_maps]
        concat_in = [
            _np.concatenate([per_core[c][i] for c in range(N_CORES)], axis=0)
            for i in range(n_params)
        ]
        concat_zeros = [
            _np.zeros((N_CORES * z.shape[0], *z.shape[1:]), z.dtype)
            for z in zero_outs
        ]
        dev_in = [jax.device_put(a, sharding)
                  for a in concat_in + concat_zeros]
        jax.block_until_ready(dev_in)
        return dev_in

    def execute(dev_in):
        out = sharded(*dev_in)
        jax.block_until_ready(out)
        return out

    def fetch(out):
        return [
            {nm: _np.asarray(out[i]).reshape(N_CORES, *out_avals[i].shape)[c]
             for i, nm in enumerate(out_names)}
            for c in range(N_CORES)
        ]

    def run(in_maps):
        return fetch(execute(stage(in_maps)))

    run.stage = stage
    run.execute = execute
    run.fetch = fetch
    return run


def _prep(lstm_output, cls_len, wq, bq, wk, bk, wv, bv, wf, bf, qk_np, fc_np):
    """Host-side prep: sample->slot assignment + per-core input maps."""
    x = np.asarray(lstm_output, dtype=np.float32)
    cls = np.asarray(cls_len).astype(np.int64)
    order = np.argsort(-cls, kind="stable")
    slots = [order[:N_CORES], order[N_CORES:]]
    ns = tuple(
        int(math.ceil(int(cls[sl].max()) / P)) if len(sl) else 0
        for sl in slots)

    def _swz(w, npdt):
        # w [o, i] -> w.T [i, o] -> [j, p, g, c]: block j holds output cols
        # j*128..(j+1)*128 for all 8 input chunks, partition-major
        wt = np.asarray(w, np.float32).T.reshape(G, P, G, P)  # [g, p, j, c]
        return np.ascontiguousarray(wt.transpose(2, 1, 0, 3)).astype(npdt)

    wqT = _swz(wq, qk_np)
    wkT = _swz(wk, qk_np)
    wvT = np.asarray(wv, np.float32).T.astype(qk_np)
    wfT = np.asarray(wf, np.float32).T.astype(fc_np)
    bq2 = np.asarray(bq, np.float32).reshape(G, P)
    bk2 = np.asarray(bk, np.float32).reshape(G, P)
    bv1 = np.asarray(bv, np.float32).reshape(1, H)
    bf1 = np.asarray(bf, np.float32).reshape(1, H)

    idx = np.arange(S)
    in_maps = []
    assign = []  # (core, slot) -> sample
    for c in range(N_CORES):
        samples = [int(slots[0][c]), int(slots[1][c])]
        assign.append(samples)
        xc = np.concatenate([x[b] for b in samples], axis=0)  # [T, H]
        xTc = np.ascontiguousarray(xc.T).astype(qk_np)  # [H, T]
        kb = np.zeros((B_LOC, 4, P), np.float32)
        for s, b in enumerate(samples):
            L = int(cls[b])
            kb[s] = np.where(idx < L, 0.0, NEG).reshape(4, P)
        in_maps.append({
            "xT": xTc, "wqT": wqT, "wkT": wkT, "wvT": wvT, "wfT": wfT,
            "bq2": bq2, "bk2": bk2, "bv1": bv1, "bf1": bf1,
            "kbias": kb,
        })
    return in_maps, assign, ns, cls, np.asarray(bf, np.float32)


def _gather(results, assign, cls, bf):
    out = np.empty((B, S, H), np.float32)
    for c in range(N_CORES):
        yc = results[c]["y"]  # [T, H]
        for s, b in enumerate(assign[c]):
            out[b] = yc[s * S:(s + 1) * S]
            L = int(cls[b])
            out[b, L:, :] = bf  # rows at/after cls_len are exactly the fc bias
    return out


def kernel(lstm_output, cls_len, wq, bq, wk, bk, wv, bv, wf, bf):
    import ml_dtypes
    qk_np = ml_dtypes.bfloat16
    fc_np = ml_dtypes.bfloat16

    in_maps, assign, ns, cls, bf_np = _prep(
        lstm_output, cls_len, wq, bq, wk, bk, wv, bv, wf, bf, qk_np, fc_np)

    key = ("run", ns)
    if key not in _CACHE:
        nc = _build(ns)
        _CACHE[key] = _make_runner(nc)
    run = _CACHE[key]
    results = run(in_maps)
    return _gather(results, assign, cls, bf_np)



# revision 21
# speedup vs baseline: 1.1601x; 1.1601x over previous
"""Trainium2 Bass kernel for ragged multi-head attention (B=16,S=512,H=1024,NH=16).

Sharding: data-parallel over batch — 16 samples over 8 cores, 2 samples/core.
Per core, everything is computed in a "transposed activation" layout so no
on-chip transposes are ever needed (see kernel_v1 docstring for the base
layout).  v2 restructures the schedule so all five engines stay busy:

  - V projection runs first (bias via a K=1 ones-row matmul; the ragged key
    mask is folded into V by multiplying each token row with a 0/1 validity
    scalar — so the softmax needs NO mask bias at all, and padded keys
    contribute exactly 0 to both the numerator and the ones-column
    denominator).
  - One j-loop interleaves the Q/K projection of head-pair block j with the
    attention of head-pair j-? (scores -> exp -> PV -> normalize), so the
    scalar-engine exp work hides under the projection matmuls.
  - scores for the two heads of a pair run in disjoint PE row groups
    (partition offsets 0/64) into one [128, 2, 512] PSUM tile, and a single
    activation computes exp over both heads at once (no per-key bias needed).
  - softmax normalization: denominators (ones-column PV rows) are packed into
    a [4, 512] tile by GPSIMD, inverted in two batched scalar-engine ops
    (1/d = exp(-ln(d)); DVE `reciprocal` is an iterative divide and costs
    ~2us per call), broadcast by GPSIMD, and applied as one bf16 DVE multiply.
  - K projection bias is dropped entirely: it only shifts each query's score
    row by a per-query constant, which softmax cancels.
  - FC bias via ones-row matmul; FC results evacuate on the scalar engine
    (idle during FC) and ship as bf16.

Ragged: per-sample lengths known on host; samples sorted by length into 2
"slots" (slot n-of-tiles = max over the 8 cores' samples in that slot), and
all per-token loops only cover ceil(L/128) 128-token tiles. Rows at/after
cls_len are exactly `bf` in the reference and are filled on the host.
"""

import math

import numpy as np

B, S, H, NH, DH = 16, 512, 1024, 16, 64
N_CORES = 8
B_LOC = B // N_CORES  # 2
T = B_LOC * S  # 1024 tokens per core
P = 128
G = H // P  # 8 contraction chunks

_CACHE = {}


def _build(ns, loop_iters=None, dtcfg=None,
           phases=("qk", "v", "sc", "pv", "fc"), debug_taps=False):
    """Build the Bass program for per-slot tile counts ns=(n0,n1).

    loop_iters: if not None, wrap the whole body in a hardware For_i loop
    (used only for benchmarking).  phases: subset of stages to emit
    (benchmark ablation only).  Returns the compiled nc.
    """
    phases = frozenset(phases)
    import concourse.bass as bass
    import concourse.mybir as mybir
    import concourse.tile as tile
    from concourse import bacc

    dt = mybir.dt
    cfg = dict(qk=dt.bfloat16, pv=dt.bfloat16, fc=dt.bfloat16)
    if dtcfg:
        cfg.update(dtcfg)
    FQK = cfg["qk"]
    FPV = cfg["pv"]
    FFC = cfg["fc"]

    nc = bacc.Bacc("TRN2", target_bir_lowering=False, debug=False,
                   num_devices=N_CORES)

    # Pin every activation (Exp, Ln, Copy) to the one table set that holds
    # them all: the default greedy placement alternates exp_and_others /
    # ln sets, inserting a ~1.3us table reload per switch (17 per pass).
    # Set ids must keep matching act_info.json order, so entries are kept
    # in order with non-target sets emptied.
    import concourse.bacc as bacc_mod
    import concourse.hw_specs as hw_specs
    _orig_tables = hw_specs.get_activation_tables(nc.m.arch)
    if "natural_log_exp_and_others" in _orig_tables:
        _patched = {k: (v if k == "natural_log_exp_and_others" else set())
                    for k, v in _orig_tables.items()}
        bacc_mod.get_activation_tables = lambda arch: _patched

    f32 = dt.float32
    xT = nc.dram_tensor("xT", [H, T], FQK, kind="ExternalInput")
    # wq/wk host-swizzled to [j, p, g, c]: column-block j is one contiguous
    # 256KB read with 2KB runs (strided reads of 256B pay a 2x DMA penalty)
    wqT = nc.dram_tensor("wqT", [G, P, G, P], FQK, kind="ExternalInput")
    wkT = nc.dram_tensor("wkT", [G, P, G, P], FQK, kind="ExternalInput")
    wvT = nc.dram_tensor("wvT", [H, H], FQK, kind="ExternalInput")
    wfT = nc.dram_tensor("wfT", [H, H], FFC, kind="ExternalInput")
    bq2 = nc.dram_tensor("bq2", [G, P], f32, kind="ExternalInput")
    bvr = nc.dram_tensor("bvr", [1, H], FQK, kind="ExternalInput")
    bfr = nc.dram_tensor("bfr", [1, H], FFC, kind="ExternalInput")
    valf = nc.dram_tensor("valf", [B_LOC * 4, P], f32, kind="ExternalInput")
    valh = nc.dram_tensor("valh", [B_LOC * 4 * NH, P], FPV, kind="ExternalInput")
    y = nc.dram_tensor("y", [T, H], FFC, kind="ExternalOutput")
    dbg = {}
    if debug_taps:
        dbg["q"] = nc.dram_tensor("dbg_q", [P, G, T], FQK, kind="ExternalOutput")
        dbg["k"] = nc.dram_tensor("dbg_k", [P, G, T], FQK, kind="ExternalOutput")
        dbg["vaug"] = nc.dram_tensor("dbg_vaug", [P, 2 * 4, NH * 65], FPV,
                                     kind="ExternalOutput")
        dbg["ao"] = nc.dram_tensor("dbg_ao", [P, G, T], FFC,
                                   kind="ExternalOutput")
        dbg["den"] = nc.dram_tensor("dbg_den", [G, P, 512], FPV,
                                    kind="ExternalOutput")
        dbg["rinv"] = nc.dram_tensor("dbg_rinv", [G, P, 512], FPV,
                                     kind="ExternalOutput")
        dbg["pt"] = nc.dram_tensor("dbg_pt", [G, P, 2, 512], FPV,
                                   kind="ExternalOutput")

    HA = 65  # per-head V columns incl. ones column
    NT = sum(ns)  # total live token tiles
    tgs = [s * 4 + tt for s in range(B_LOC) for tt in range(ns[s])]
    slot_order = sorted(range(B_LOC), key=lambda s: ns[s])  # small first

    with tile.TileContext(nc) as tc:
        import contextlib
        ctx = contextlib.ExitStack()
        with ctx:
            const = ctx.enter_context(tc.tile_pool(name="const", bufs=1))
            wqk_pool = ctx.enter_context(tc.tile_pool(name="wqk", bufs=3))
            pt_pool = ctx.enter_context(tc.tile_pool(name="pt", bufs=6))
            raw_pool = ctx.enter_context(tc.tile_pool(name="raw", bufs=6))
            den_pool = ctx.enter_context(tc.tile_pool(name="den", bufs=2))
            rinv_pool = ctx.enter_context(tc.tile_pool(name="rinv", bufs=4))  # rln+rinv x2
            rb_pool = ctx.enter_context(tc.tile_pool(name="rb", bufs=4))
            rrow_pool = ctx.enter_context(tc.tile_pool(name="rrow", bufs=4))
            out_pool = ctx.enter_context(tc.tile_pool(name="out", bufs=4))
            ps_mm = ctx.enter_context(tc.tile_pool(name="psmm", bufs=2, space="PSUM"))
            ps_sc = ctx.enter_context(tc.tile_pool(name="pssc", bufs=2, space="PSUM"))
            ps_pv = ctx.enter_context(tc.tile_pool(name="pspv", bufs=2, space="PSUM"))

            # resident tensors
            xT_sb = const.tile([P, G, T], FQK)
            qT_sb = const.tile([P, G, T], FQK)
            kT_sb = const.tile([P, G, T], FQK)
            vaug_sb = const.tile([P, 2 * 4, NH * HA], FPV)
            aoT_sb = const.tile([P, G, T], FFC)
            wv_sb = const.tile([P, G, H], FQK)
            wf_sb = const.tile([P, G, H], FFC)
            bq_sb = const.tile([P, G], f32)
            bvr_sb = const.tile([1, H], FQK)
            bfr_sb = const.tile([1, H], FFC)
            val_sb = const.tile([P, B_LOC, 4], f32)
            ones_sb = const.tile([1, P], FQK)

            def body():
                # ---- preload ----
                for g in range(G):
                    eng = nc.sync if g % 2 == 0 else nc.scalar
                    eng.dma_start(
                        out=xT_sb[:, g, :],
                        in_=xT.ap()[g * P:(g + 1) * P, :])
                nc.scalar.dma_start(
                    out=bq_sb[:], in_=bq2.ap().rearrange("g p -> p g"))
                nc.scalar.dma_start(out=bvr_sb[:], in_=bvr.ap())
                nc.scalar.dma_start(out=bfr_sb[:], in_=bfr.ap())
                nc.scalar.dma_start(
                    out=val_sb[:], in_=valf.ap().rearrange("(s k) p -> p s k",
                                                           s=B_LOC))
                # ones columns of V_aug = per-key 0/1 validity (bf16 copy)
                nc.scalar.dma_start(
                    out=vaug_sb.rearrange(
                        "p t (h c) -> p t h c", c=HA)[:, :, :, 64:65],
                    in_=valh.ap().rearrange("th p -> p th"))
                nc.scalar.dma_start(
                    out=wv_sb[:],
                    in_=wvT.ap().rearrange("(g p) o -> p g o", p=P))
                nc.scalar.dma_start(
                    out=wf_sb[:],
                    in_=wfT.ap().rearrange("(g p) o -> p g o", p=P))
                nc.vector.memset(ones_sb[:], 1.0)

                # ---- V projection (natural layout, into V_aug) ----
                if "v" in phases:
                    for ob in range(2):  # 512-wide output column blocks
                        for s in range(B_LOC):
                            for tt in range(ns[s]):
                                tg = s * 4 + tt
                                ps = ps_mm.tile([P, 512], f32, tag="psmm")
                                for g in range(G):
                                    nc.tensor.matmul(
                                        ps[:],
                                        lhsT=xT_sb[:, g, tg * P:(tg + 1) * P],
                                        rhs=wv_sb[:, g, ob * 512:(ob + 1) * 512],
                                        start=(g == 0), stop=False)
                                nc.tensor.matmul(
                                    ps[:], lhsT=ones_sb[:],
                                    rhs=bvr_sb[:, ob * 512:(ob + 1) * 512],
                                    start=False, stop=True)
                                vdst = vaug_sb[:, tg, :].rearrange(
                                    "p (h c) -> p h c",
                                    c=HA)[:, ob * 8:(ob + 1) * 8, 0:64]
                                nc.scalar.activation(
                                    out=vdst,
                                    in_=ps[:].rearrange("p (h c) -> p h c",
                                                        c=64),
                                    func=mybir.ActivationFunctionType.Copy,
                                    scale=val_sb[:, s, tt:tt + 1])

                # ---- fused Q/K projection + attention j-loop ----
                def proj_j(wT_d, j, dst, bias):
                    wblk = wqk_pool.tile([P, G, P], FQK, tag="wqk")
                    nc.sync.dma_start(out=wblk[:], in_=wT_d.ap()[j])
                    pss = {}
                    for s in range(B_LOC):
                        if ns[s]:
                            pss[s] = ps_mm.tile([P, 512], f32, tag="psmm",
                                                name=f"psqk{s}")
                    for g in range(G):
                        for s, ps in pss.items():
                            nc.tensor.matmul(
                                ps[:, :ns[s] * P], lhsT=wblk[:, g, :],
                                rhs=xT_sb[:, g, s * S:s * S + ns[s] * P],
                                start=(g == 0), stop=(g == G - 1))
                    for s, ps in pss.items():
                        W = ns[s] * P
                        if bias is not None:
                            nc.vector.tensor_scalar_add(
                                out=dst[:, j, s * S:s * S + W], in0=ps[:, :W],
                                scalar1=bias[:, j:j + 1])
                        else:
                            nc.vector.tensor_copy(
                                out=dst[:, j, s * S:s * S + W], in_=ps[:, :W])

                def attn_j(g2):
                    """scores+exp+PV+normalize for head pair g2, both slots."""
                    pairs = []  # (s, hh, raw, W)
                    for s in slot_order:
                        n = ns[s]
                        if n == 0 or "sc" not in phases:
                            continue
                        W = n * P
                        s0 = s * S
                        pts = []
                        for kt in range(n):
                            ps = ps_sc.tile([P, 2, 512], f32, tag="pssc")
                            for hh in (0, 1):
                                po = hh * 64
                                nc.tensor.matmul(
                                    ps[:, hh, :W],
                                    lhsT=kT_sb[po:po + 64, g2,
                                               s0 + kt * P:s0 + (kt + 1) * P],
                                    rhs=qT_sb[po:po + 64, g2, s0:s0 + W],
                                    start=True, stop=True)
                            pt = pt_pool.tile([P, 2, 512], FPV, tag="pt")
                            nc.scalar.activation(
                                out=pt[:, :, :W], in_=ps[:, :, :W],
                                func=mybir.ActivationFunctionType.Exp,
                                scale=0.125)
                            pts.append(pt)
                        if "pv" not in phases:
                            continue
                        for hh in (0, 1):
                            h = 2 * g2 + hh
                            pv = ps_pv.tile([HA, 512], f32, tag="pspv")
                            for kt in range(n):
                                nc.tensor.matmul(
                                    pv[:, :W],
                                    lhsT=vaug_sb[:, s * 4 + kt,
                                                 HA * h:HA * h + HA],
                                    rhs=pts[kt][:, hh, :W],
                                    start=(kt == 0), stop=(kt == n - 1))
                            raw = raw_pool.tile([HA, 512], FPV, tag="raw")
                            nc.vector.tensor_copy(out=raw[:, :W],
                                                  in_=pv[:, :W])
                            pairs.append((s, hh, raw, W))
                    if not pairs:
                        return
                    # denominators -> 32-partition replicated groups of one
                    # tile; batched 1/d = exp(-ln(d)); apply in 32-row halves
                    # NOTE: gpsimd partition_broadcast writes garbage for
                    # output base partitions != 0 on HW (verified), so the
                    # denominators are packed into 32-spaced rows with tiny
                    # SBUF->SBUF DMAs instead.
                    den = den_pool.tile([128, 512], FPV, tag="den")
                    nc.vector.memset(den[:], 1.0)
                    for i, (s, hh, raw, W) in enumerate(pairs):
                        nc.sync.dma_start(out=den[32 * i:32 * i + 1, :W],
                                          in_=raw[64:65, :W])
                    npair = len(pairs)
                    rln = rinv_pool.tile([128, 512], f32, tag="rln")
                    nc.scalar.activation(
                        out=rln[0:32 * npair, :], in_=den[0:32 * npair, :],
                        func=mybir.ActivationFunctionType.Ln)
                    rinv = rinv_pool.tile([128, 512], FPV, tag="rinv")
                    nc.scalar.activation(
                        out=rinv[0:32 * npair, :], in_=rln[0:32 * npair, :],
                        func=mybir.ActivationFunctionType.Exp, scale=-1.0)
                    if debug_taps:
                        nc.sync.dma_start(out=dbg["den"].ap()[g2],
                                          in_=den[:])
                        nc.sync.dma_start(out=dbg["rinv"].ap()[g2],
                                          in_=rinv[:])
                        nc.sync.dma_start(out=dbg["pt"].ap()[g2],
                                          in_=pts[0][:])
                    for i, (s, hh, raw, W) in enumerate(pairs):
                        # partition_broadcast is only HW-correct reading
                        # partition 0 / writing base 0, so bounce the pair's
                        # rinv row into a base-0 [1,W] tile via SBUF DMA.
                        rrow = rrow_pool.tile([1, 512], FPV, tag="rrow")
                        nc.scalar.dma_start(out=rrow[0:1, :W],
                                            in_=rinv[32 * i:32 * i + 1, :W])
                        rb = rb_pool.tile([64, 512], FPV, tag="rb")
                        nc.gpsimd.partition_broadcast(rb[:, :W],
                                                      rrow[0:1, :W])
                        nc.vector.tensor_mul(
                            aoT_sb[hh * 64:hh * 64 + 64, g2,
                                   s * S:s * S + W],
                            raw[0:64, :W], rb[:, :W])

                for j in range(G):
                    if "qk" in phases:
                        proj_j(wqT, j, qT_sb, bq_sb)
                        proj_j(wkT, j, kT_sb, None)
                    attn_j(j)

                # ---- FC ----
                def fc_slot(s):
                    for oh in range(2 if "fc" in phases else 0):
                        for tt in range(ns[s]):
                            tg = s * 4 + tt
                            ps = ps_mm.tile([P, 512], f32, tag="psmm")
                            for g in range(G):
                                nc.tensor.matmul(
                                    ps[:],
                                    lhsT=aoT_sb[:, g, tg * P:(tg + 1) * P],
                                    rhs=wf_sb[:, g, oh * 512:(oh + 1) * 512],
                                    start=(g == 0), stop=False)
                            nc.tensor.matmul(
                                ps[:], lhsT=ones_sb[:],
                                rhs=bfr_sb[:, oh * 512:(oh + 1) * 512],
                                start=False, stop=True)
                            ot = out_pool.tile([P, 512], FFC, tag="out")
                            nc.scalar.copy(out=ot[:], in_=ps[:])
                            nc.sync.dma_start(
                                out=y.ap()[tg * P:(tg + 1) * P,
                                           oh * 512:(oh + 1) * 512],
                                in_=ot[:])

                for s in slot_order:
                    fc_slot(s)

                if debug_taps:
                    nc.sync.dma_start(out=dbg["q"].ap(), in_=qT_sb[:])
                    nc.sync.dma_start(out=dbg["k"].ap(), in_=kT_sb[:])
                    nc.sync.dma_start(out=dbg["vaug"].ap(), in_=vaug_sb[:])
                    nc.sync.dma_start(out=dbg["ao"].ap(), in_=aoT_sb[:])

            if loop_iters is None:
                body()
            else:
                with tc.For_i(0, loop_iters, 1,
                              hint_engines=tuple(mybir.ALL_ENGINES)):
                    body()

    nc.compile()
    return nc


def _make_runner(nc):
    """Compile nc into a reusable 8-core jitted callable (axon PJRT path)."""
    import jax
    import numpy as _np
    from jax.experimental.shard_map import shard_map
    from jax.sharding import Mesh, NamedSharding, PartitionSpec

    import concourse.mybir as mybir
    from concourse import bass2jax

    bass2jax.install_neuronx_cc_hook()
    partition_name = (nc.partition_id_tensor.name
                      if nc.partition_id_tensor else None)
    in_names, out_names, out_avals, zero_outs = [], [], [], []
    for alloc in nc.m.functions[0].allocations:
        if not isinstance(alloc, mybir.MemoryLocationSet):
            continue
        name = alloc.memorylocations[0].name
        if alloc.kind == "ExternalInput":
            if name != partition_name:
                in_names.append(name)
        elif alloc.kind == "ExternalOutput":
            shape = tuple(alloc.tensor_shape)
            dtype = mybir.dt.np(alloc.dtype)
            out_names.append(name)
            out_avals.append(jax.core.ShapedArray(shape, dtype))
            zero_outs.append(_np.zeros(shape, dtype))
    n_params = len(in_names)
    in_names_all = in_names + out_names
    if partition_name is not None:
        in_names_all.append(partition_name)

    def _body(*args):
        operands = list(args)
        if partition_name is not None:
            operands.append(bass2jax.partition_id_tensor())
        outs = bass2jax._bass_exec_p.bind(
            *operands, out_avals=tuple(out_avals),
            in_names=tuple(in_names_all), out_names=tuple(out_names),
            lowering_input_output_aliases=(),
            sim_require_finite=False, sim_require_nnan=False, nc=nc)
        return tuple(outs)

    devices = jax.devices()[:N_CORES]
    mesh = Mesh(np.asarray(devices), ("core",))
    nio = n_params + len(out_names)
    sharded = jax.jit(
        shard_map(_body, mesh=mesh,
                  in_specs=(PartitionSpec("core"),) * nio,
                  out_specs=(PartitionSpec("core"),) * len(out_names),
                  check_rep=False),
        keep_unused=True)
    sharding = NamedSharding(mesh, PartitionSpec("core"))

    def stage(in_maps):
        per_core = [[_np.asarray(m[nm]) for nm in in_names] for m in in_maps]
        concat_in = [
            _np.concatenate([per_core[c][i] for c in range(N_CORES)], axis=0)
            for i in range(n_params)
        ]
        concat_zeros = [
            _np.zeros((N_CORES * z.shape[0], *z.shape[1:]), z.dtype)
            for z in zero_outs
        ]
        dev_in = [jax.device_put(a, sharding)
                  for a in concat_in + concat_zeros]
        jax.block_until_ready(dev_in)
        return dev_in

    def execute(dev_in):
        out = sharded(*dev_in)
        jax.block_until_ready(out)
        return out

    def fetch(out):
        return [
            {nm: _np.asarray(out[i]).reshape(N_CORES, *out_avals[i].shape)[c]
             for i, nm in enumerate(out_names)}
            for c in range(N_CORES)
        ]

    def run(in_maps):
        return fetch(execute(stage(in_maps)))

    run.stage = stage
    run.execute = execute
    run.fetch = fetch
    return run


def _prep(lstm_output, cls_len, wq, bq, wk, bk, wv, bv, wf, bf, qk_np, fc_np):
    """Host-side prep: sample->slot assignment + per-core input maps."""
    x = np.asarray(lstm_output, dtype=np.float32)
    cls = np.asarray(cls_len).astype(np.int64)
    order = np.argsort(-cls, kind="stable")
    slots = [order[:N_CORES], order[N_CORES:]]
    ns = tuple(
        int(math.ceil(int(cls[sl].max()) / P)) if len(sl) else 0
        for sl in slots)

    def _swz(w, npdt):
        # w [o, i] -> w.T [i, o] -> [j, p, g, c]: block j holds output cols
        # j*128..(j+1)*128 for all 8 input chunks, partition-major
        wt = np.asarray(w, np.float32).T.reshape(G, P, G, P)  # [g, p, j, c]
        return np.ascontiguousarray(wt.transpose(2, 1, 0, 3)).astype(npdt)

    wqT = _swz(wq, qk_np)
    wkT = _swz(wk, qk_np)
    wvT = np.asarray(wv, np.float32).T.astype(qk_np)
    wfT = np.asarray(wf, np.float32).T.astype(fc_np)
    bq2 = np.asarray(bq, np.float32).reshape(G, P)
    bvr = np.asarray(bv, np.float32).reshape(1, H).astype(qk_np)
    bfr = np.asarray(bf, np.float32).reshape(1, H).astype(fc_np)

    idx = np.arange(S)
    in_maps = []
    assign = []  # (core, slot) -> sample
    for c in range(N_CORES):
        samples = [int(slots[0][c]), int(slots[1][c])]
        assign.append(samples)
        xc = np.concatenate([x[b] for b in samples], axis=0)  # [T, H]
        xTc = np.ascontiguousarray(xc.T).astype(qk_np)  # [H, T]
        vf = np.zeros((B_LOC * 4, P), np.float32)
        for s, b in enumerate(samples):
            L = int(cls[b])
            vf[s * 4:(s + 1) * 4] = (idx < L).astype(
                np.float32).reshape(4, P)
        in_maps.append({
            "xT": xTc, "wqT": wqT, "wkT": wkT, "wvT": wvT, "wfT": wfT,
            "bq2": bq2, "bvr": bvr, "bfr": bfr, "valf": vf,
            "valh": np.repeat(vf, NH, axis=0).astype(qk_np),
        })
    return in_maps, assign, ns, cls, np.asarray(bf, np.float32)


def _gather(results, assign, cls, bf):
    out = np.empty((B, S, H), np.float32)
    for c in range(N_CORES):
        yc = np.asarray(results[c]["y"], np.float32)  # [T, H]
        for s, b in enumerate(assign[c]):
            out[b] = yc[s * S:(s + 1) * S]
            L = int(cls[b])
            out[b, L:, :] = bf  # rows at/after cls_len are exactly the fc bias
    return out


def kernel(lstm_output, cls_len, wq, bq, wk, bk, wv, bv, wf, bf):
    import ml_dtypes
    qk_np = ml_dtypes.bfloat16
    fc_np = ml_dtypes.bfloat16

    in_maps, assign, ns, cls, bf_np = _prep(
        lstm_output, cls_len, wq, bq, wk, bk, wv, bv, wf, bf, qk_np, fc_np)

    key = ("run", ns)
    if key not in _CACHE:
        nc = _build(ns)
        _CACHE[key] = _make_runner(nc)
    run = _CACHE[key]
    results = run(in_maps)
    return _gather(results, assign, cls, bf_np)
